# revision 6
# baseline (speedup 1.0000x reference)
"""Causal MHA kernel for TRN2, tensor-parallel over heads across 8 NeuronCores.

Sharding: core i computes heads {2i, 2i+1} fully (q/k/v projection, causal
attention, output-projection partial product); the 8 partial outputs are
summed on the host (out = sum_h attn_h @ Wo_h.T decomposes per head group).

v3 changes vs v2:
  - Exact causal trimming: scores/exp/PV run per 128-wide k-tile with the
    dead q-prefix of diagonal tiles skipped (partial-width matmuls).
  - Output projection in fp8 DoubleRow: attn split hi/lo e4m3 on device,
    Wo split hi/lo on host; 3 DR matmuls contract all 256 local dims at
    0.5 cyc/row (vs 2 fp16 matmuls at 1 cyc/row).
  - Softmax denominator accumulation moved from DVE to the idle Pool
    engine; single f16 ones-matmul per (chunk, head) broadcasts the sum.
  - Software pipelining: projection matmul groups of chunk N+1 are emitted
    interleaved into the attention instruction stream of chunk N, so the
    PE never idles waiting on the Act/DVE softmax chain.
  - First x-chunk DMA issued before weight DMAs (separate queues) to cut
    the startup stall.
"""
import numpy as np
import ml_dtypes

import concourse.bass as bass
import concourse.mybir as mybir
import concourse.tile as tile
from concourse import bacc
from concourse.bass_utils import run_bass_kernel_spmd

B, S, D = 4, 2048, 2048
H, DK = 16, 128
NCORES = 8
HPC = H // NCORES          # heads per core
CD = HPC * DK              # 256 contraction dims per core in out-proj
SC = 512                   # q/s chunk
NSC = S // SC              # 4
NST = SC // 128            # s-tiles per chunk
NDB = D // 256             # 8 d-pair-chunks (DoubleRow contraction tiles)
F32 = mybir.dt.float32
F16 = mybir.dt.float16
F8 = mybir.dt.float8e4
DR = mybir.MatmulPerfMode.DoubleRow
EXPF = mybir.ActivationFunctionType.Exp

WSCALE = 64.0              # host-side scale on Wq/Wk/Wv for fp8 range
SCALE = (1.0 / np.sqrt(DK)) / (WSCALE * WSCALE)
ASCALE = 16.0              # attn stored as 16*attn in fp8 hi/lo
WOS = 1024.0               # host-side scale on Wo for fp8 range
# bc = cst @ den: cst holds ASCALE/WSCALE so attps*rbc = ASCALE*attn
CSTV = WSCALE / ASCALE


def build_nc():
    nc = bacc.Bacc(None)
    # x hi/lo fp8: element (b, k, dcb, i, hl, s) = fp8_hl(x[b, s, dcb*256+i*128+k])
    xT = nc.dram_tensor("xT", [B, 128, NDB, 2, 2, S], F8, kind="ExternalInput")
    # wXa: (k, dcb, i, j, o) = Wh[dcb*256+i*128+k, o]  (duplicated over j)
    # wXb: (k, dcb, i, o)    = Wl[dcb*256+i*128+k, o]
    wqa = nc.dram_tensor("wqa", [128, NDB, 2, 2, CD], F8, kind="ExternalInput")
    wqb = nc.dram_tensor("wqb", [128, NDB, 2, CD], F8, kind="ExternalInput")
    wka = nc.dram_tensor("wka", [128, NDB, 2, 2, CD], F8, kind="ExternalInput")
    wkb = nc.dram_tensor("wkb", [128, NDB, 2, CD], F8, kind="ExternalInput")
    wva = nc.dram_tensor("wva", [128, NDB, 2, 2, CD], F8, kind="ExternalInput")
    wvb = nc.dram_tensor("wvb", [128, NDB, 2, CD], F8, kind="ExternalInput")
    # wo8[dk, hh, o] = Wo[o, hh*128+dk] * WOS, hi/lo fp8 parts
    wo8a = nc.dram_tensor("wo8a", [128, HPC, D], F8, kind="ExternalInput")
    wo8b = nc.dram_tensor("wo8b", [128, HPC, D], F8, kind="ExternalInput")
    # msk[k, j, q] = (j*128 + k) <= q  for the 4 diagonal tile offsets
    msk = nc.dram_tensor("msk", [128, NST, SC], F16, kind="ExternalInput")
    out = nc.dram_tensor("out", [B, S, D], F16, kind="ExternalOutput")

    with tile.TileContext(nc) as tc:
        with (
            tc.tile_pool(name="p_xs", bufs=2) as p_xs,
            tc.tile_pool(name="p_kv", bufs=2) as p_kv,
            tc.tile_pool(name="p_q", bufs=4) as p_q,
            tc.tile_pool(name="p_pt", bufs=2) as p_pt,
            tc.tile_pool(name="p_w", bufs=1) as p_w,
            tc.tile_pool(name="p_sm", bufs=2) as p_sm,
            tc.tile_pool(name="p_a8", bufs=2) as p_a8,
            tc.tile_pool(name="p_osb", bufs=2) as p_osb,
            tc.tile_pool(name="ps_m", bufs=4, space="PSUM") as ps_m,
            tc.tile_pool(name="ps_a", bufs=1, space="PSUM") as ps_a,
            tc.tile_pool(name="ps_bc", bufs=1, space="PSUM") as ps_bc,
            tc.tile_pool(name="ps_o", bufs=2, space="PSUM") as ps_o,
        ):
            # ---- weight / constant loads (gpsimd queue; x stream on sync) --
            wa_sb = {}
            wb_sb = {}
            wa_sb["q"] = p_w.tile([128, NDB, 2, 2, CD], F8, tag="wqa", name="wqa_sb")
            wb_sb["q"] = p_w.tile([128, NDB, 2, CD], F8, tag="wqb", name="wqb_sb")
            wa_sb["k"] = p_w.tile([128, NDB, 2, 2, CD], F8, tag="wka", name="wka_sb")
            wb_sb["k"] = p_w.tile([128, NDB, 2, CD], F8, tag="wkb", name="wkb_sb")
            wa_sb["v"] = p_w.tile([128, NDB, 2, 2, CD], F8, tag="wva", name="wva_sb")
            wb_sb["v"] = p_w.tile([128, NDB, 2, CD], F8, tag="wvb", name="wvb_sb")
            wo_sb = {}
            wo_sb["a"] = p_w.tile([128, HPC, D], F8, tag="wo8a", name="wo8a_sb")
            wo_sb["b"] = p_w.tile([128, HPC, D], F8, tag="wo8b", name="wo8b_sb")
            msk_sb = p_w.tile([128, NST, SC], F16, tag="msk")
            cst_sb = p_w.tile([128, 128], F16, tag="cst")
            nc.gpsimd.dma_start(out=wa_sb["q"], in_=wqa[:])
            nc.gpsimd.dma_start(out=wb_sb["q"], in_=wqb[:])
            nc.gpsimd.dma_start(out=wa_sb["k"], in_=wka[:])
            nc.gpsimd.dma_start(out=wb_sb["k"], in_=wkb[:])
            nc.gpsimd.dma_start(out=wa_sb["v"], in_=wva[:])
            nc.gpsimd.dma_start(out=wb_sb["v"], in_=wvb[:])
            nc.gpsimd.dma_start(out=wo_sb["a"], in_=wo8a[:])
            nc.gpsimd.dma_start(out=wo_sb["b"], in_=wo8b[:])
            nc.gpsimd.dma_start(out=msk_sb, in_=msk[:])
            nc.vector.memset(cst_sb, CSTV)

            xs_t = {}      # (b, sc) -> xs tile
            qT_t = {}      # (b, sc) -> qT tile
            kT_t = {}      # b -> kT tile
            v_t = {}       # b -> v tile

            def emit_xs(b, sc):
                xs = p_xs.tile([128, NDB, 2, 2, SC], F8, tag="xs")
                nc.sync.dma_start(
                    out=xs, in_=xT[b][:, :, :, :, sc * SC:(sc + 1) * SC])
                xs_t[(b, sc)] = xs

            def proj_units(b, sc):
                """8 emission closures: 4 QK groups + 4 V groups."""
                if b >= B:
                    return []
                units = []
                if b not in kT_t:
                    kT_t[b] = p_kv.tile([128, HPC, S], F16, tag="kT", name="kT")
                    v_t[b] = p_kv.tile([128, NSC * NST, CD], F16, tag="v", name="v_sb")
                kT = kT_t[b]
                v_sb = v_t[b]
                qT = p_q.tile([128, HPC, SC], F16, tag="qT")
                qT_t[(b, sc)] = qT

                def qk_group(h, wn):
                    def emit():
                        xs = xs_t[(b, sc)]
                        ps = ps_m.tile([128, SC], F32, tag="ps")
                        for dcb in range(NDB):
                            for i in range(2):
                                nc.tensor.matmul(
                                    ps,
                                    wa_sb[wn][:, dcb, i, :,
                                              h * DK:(h + 1) * DK],
                                    xs[:, dcb, i, :, :],
                                    start=(dcb == 0 and i == 0),
                                    stop=False,
                                    perf_mode=DR,
                                )
                        for dcb in range(NDB):
                            nc.tensor.matmul(
                                ps,
                                wb_sb[wn][:, dcb, :, h * DK:(h + 1) * DK],
                                xs[:, dcb, :, 0, :],
                                start=False,
                                stop=(dcb == NDB - 1),
                                perf_mode=DR,
                            )
                        if wn == "q":
                            nc.vector.tensor_copy(qT[:, h, :], ps)
                        else:
                            nc.vector.tensor_copy(
                                kT[:, h, sc * SC:(sc + 1) * SC], ps)
                    return emit

                def v_group(st):
                    def emit():
                        xs = xs_t[(b, sc)]
                        psv = ps_m.tile([128, SC], F32, tag="ps")
                        c0, c1 = st * 128, (st + 1) * 128
                        for dcb in range(NDB):
                            for i in range(2):
                                nc.tensor.matmul(
                                    psv[:, :CD],
                                    xs[:, dcb, i, :, c0:c1],
                                    wa_sb["v"][:, dcb, i, :, :],
                                    start=(dcb == 0 and i == 0),
                                    stop=False,
                                    perf_mode=DR,
                                )
                        for dcb in range(NDB):
                            nc.tensor.matmul(
                                psv[:, :CD],
                                xs[:, dcb, :, 0, c0:c1],
                                wb_sb["v"][:, dcb, :, :],
                                start=False,
                                stop=(dcb == NDB - 1),
                                perf_mode=DR,
                            )
                        nc.vector.tensor_copy(
                            v_sb[:, sc * NST + st, :], psv[:, :CD])
                    return emit

                for h in range(HPC):
                    units.append(qk_group(h, "q"))
                    units.append(qk_group(h, "k"))
                for st in range(NST):
                    units.append(v_group(st))
                return units

            def attn_head(b, c, h):
                """scores -> exp -> mask -> den(Pool) -> PV."""
                kT = kT_t[b]
                v_sb = v_t[b]
                qT = qT_t[(b, c)]
                nkt = 4 * c + 4
                pT = p_pt.tile([128, 16, SC], F16, tag="pT")
                den = p_sm.tile([128, SC], F16, tag="den")
                attps = ps_a.tile([128, SC], F32, tag="attps")
                for kt in range(nkt):
                    j = kt - 4 * c
                    qlo = j * 128 if j >= 0 else 0
                    sps = ps_m.tile([128, SC], F32, tag="ps")
                    nc.tensor.matmul(
                        sps[:, qlo:],
                        kT[:, h, kt * 128:(kt + 1) * 128],
                        qT[:, h, qlo:],
                        start=True, stop=True,
                    )
                    nc.scalar.activation(
                        pT[:, kt, qlo:], sps[:, qlo:], EXPF, scale=SCALE)
                    if j >= 0:
                        nc.vector.tensor_mul(
                            pT[:, kt, qlo:], pT[:, kt, qlo:],
                            msk_sb[:, j, qlo:])
                    if kt == 0:
                        nc.gpsimd.tensor_copy(den, pT[:, 0, :])
                    else:
                        nc.gpsimd.tensor_add(
                            den[:, qlo:], den[:, qlo:], pT[:, kt, qlo:])
                    nc.tensor.matmul(
                        attps[:, qlo:],
                        v_sb[:, kt, h * DK:(h + 1) * DK],
                        pT[:, kt, qlo:],
                        start=(kt == 0), stop=(kt == nkt - 1),
                        skip_group_check=(kt > 0),
                    )
                return den, attps

            def finish_head(den, attps):
                """bc ones-matmul + reciprocal + normalize: emitted a few
                proj units after the PV chain so the PE never waits on the
                Pool den accumulation."""
                bc = ps_bc.tile([128, SC], F32, tag="bc")
                nc.tensor.matmul(bc, cst_sb, den, start=True, stop=True)
                rbc = p_sm.tile([128, SC], F32, tag="rbc")
                t16 = p_sm.tile([128, SC], F16, tag="t16")
                nc.vector.reciprocal(rbc, bc)
                nc.vector.tensor_mul(t16, attps, rbc)
                return t16

            def attn_split(t16s, a8a, a8b):
                for h, t16 in enumerate(t16s):
                    nc.vector.tensor_copy(a8a[:, h, :], t16)
                    nc.vector.tensor_sub(a8b[:, h, :], t16, a8a[:, h, :])

            def outproj(b, c, a8a, a8b):
                for st in range(NST):
                    osb = p_osb.tile([128, NSC, SC], F16, tag="osb")
                    for oc in range(NSC):
                        ops = ps_o.tile([128, SC], F32, tag="ops")
                        os = slice(oc * SC, (oc + 1) * SC)
                        ts = slice(st * 128, (st + 1) * 128)
                        nc.tensor.matmul(ops, a8a[:, :, ts], wo_sb["a"][:, :, os],
                                         start=True, stop=False, perf_mode=DR)
                        nc.tensor.matmul(ops, a8b[:, :, ts], wo_sb["a"][:, :, os],
                                         start=False, stop=False, perf_mode=DR)
                        nc.tensor.matmul(ops, a8a[:, :, ts], wo_sb["b"][:, :, os],
                                         start=False, stop=True, perf_mode=DR)
                        if oc % 2 == 0:
                            nc.scalar.copy(osb[:, oc, :], ops)
                        else:
                            nc.vector.tensor_copy(osb[:, oc, :], ops)
                    nc.gpsimd.dma_start(
                        out=out[b,
                                (c * NST + st) * 128:(c * NST + st + 1) * 128,
                                :],
                        in_=osb,
                    )

            # ---------------- schedule ----------------
            steps = [(b, c) for b in range(B) for c in range(NSC)]
            emit_xs(0, 0)
            emit_xs(0, 1)
            for u in proj_units(0, 0):
                u()
            for i, (b, c) in enumerate(steps):
                nb_, nc_ = steps[i + 1] if i + 1 < len(steps) else (B, 0)
                units = proj_units(nb_, nc_)
                # prefetch the x chunk one step ahead of its proj units
                pf = steps[i + 2] if i + 2 < len(steps) else None
                if pf is not None:
                    emit_xs(*pf)
                den0, att0 = attn_head(b, c, 0)
                for u in units[:2]:
                    u()
                t0 = finish_head(den0, att0)
                den1, att1 = attn_head(b, c, 1)
                for u in units[2:4]:
                    u()
                t1 = finish_head(den1, att1)
                a8a = p_a8.tile([128, HPC, SC], F8, tag="a8a")
                a8b = p_a8.tile([128, HPC, SC], F8, tag="a8b")
                attn_split((t0, t1), a8a, a8b)
                for u in units[4:6]:
                    u()
                outproj(b, c, a8a, a8b)
                for u in units[6:]:
                    u()
    nc.compile()
    return nc


def _fp8_split(a):
    hi = a.astype(ml_dtypes.float8_e4m3)
    lo = (a - hi.astype(np.float32)).astype(ml_dtypes.float8_e4m3)
    return hi, lo


def prepare_in_maps(x, Wq, Wk, Wv, Wo):
    x = np.asarray(x, dtype=np.float32)
    Wq = np.asarray(Wq, dtype=np.float32)
    Wk = np.asarray(Wk, dtype=np.float32)
    Wv = np.asarray(Wv, dtype=np.float32)
    Wo = np.asarray(Wo, dtype=np.float32)

    # x -> [B, 128, NDB, 2, 2, S] fp8 hi/lo
    xT = np.ascontiguousarray(x.transpose(0, 2, 1))  # [B, D, S]
    xh, xl = _fp8_split(xT)
    xh = xh.reshape(B, NDB, 2, 128, S)
    xl = xl.reshape(B, NDB, 2, 128, S)
    x8 = np.stack([xh, xl], axis=4)          # [B, NDB, 2, 128, 2, S]
    x8 = np.ascontiguousarray(x8.transpose(0, 3, 1, 2, 4, 5))

    qf = np.arange(SC)[None, None, :]
    kg = (np.arange(NST) * 128)[None, :, None] + np.arange(128)[:, None, None]
    msk = (kg <= qf).astype(np.float16)      # [128, NST, SC]

    in_maps = []
    for c in range(NCORES):
        r0, r1 = c * CD, (c + 1) * CD
        m = {"xT": x8, "msk": msk}
        for nm, W in (("q", Wq), ("k", Wk), ("v", Wv)):
            Wm = np.ascontiguousarray(W[r0:r1].T) * WSCALE   # [D, CD]
            hi, lo = _fp8_split(Wm)
            hi = hi.reshape(NDB, 2, 128, CD).transpose(2, 0, 1, 3)
            wa = np.ascontiguousarray(
                np.broadcast_to(hi[:, :, :, None, :], (128, NDB, 2, 2, CD)))
            wb = np.ascontiguousarray(
                lo.reshape(NDB, 2, 128, CD).transpose(2, 0, 1, 3))
            m[f"w{nm}a"] = wa
            m[f"w{nm}b"] = wb
        Wo_c = np.ascontiguousarray(Wo[:, r0:r1]) * WOS      # [D, CD]
        woh, wol = _fp8_split(Wo_c)
        # [D, CD] -> [128 dk, HPC, D]
        m["wo8a"] = np.ascontiguousarray(
            woh.reshape(D, HPC, 128).transpose(2, 1, 0))
        m["wo8b"] = np.ascontiguousarray(
            wol.reshape(D, HPC, 128).transpose(2, 1, 0))
        in_maps.append(m)
    return in_maps


_NC_CACHE = None


def kernel(x, Wq, Wk, Wv, Wo):
    global _NC_CACHE
    in_maps = prepare_in_maps(x, Wq, Wk, Wv, Wo)
    if _NC_CACHE is None:
        _NC_CACHE = build_nc()
    res = run_bass_kernel_spmd(_NC_CACHE, in_maps, list(range(NCORES)))
    total = res.results[0]["out"].astype(np.float32).copy()
    for i in range(1, NCORES):
        total += res.results[i]["out"].astype(np.float32)
    return total / (ASCALE * WOS)


# revision 7
# speedup vs baseline: 1.2332x; 1.2332x over previous
"""Causal MHA kernel for TRN2, tensor-parallel over heads across 8 NeuronCores.

Sharding: core i computes heads {2i, 2i+1} fully (q/k/v projection, causal
attention, output-projection partial product); the 8 partial outputs are
summed on the host (out = sum_h attn_h @ Wo_h.T decomposes per head group).

v3 changes vs v2:
  - Exact causal trimming: scores/exp/PV run per 128-wide k-tile with the
    dead q-prefix of diagonal tiles skipped (partial-width matmuls).
  - Output projection in fp8 DoubleRow: attn split hi/lo e4m3 on device,
    Wo split hi/lo on host; 3 DR matmuls contract all 256 local dims at
    0.5 cyc/row (vs 2 fp16 matmuls at 1 cyc/row).
  - Softmax denominator accumulation moved from DVE to the idle Pool
    engine; single f16 ones-matmul per (chunk, head) broadcasts the sum.
  - Software pipelining: projection matmul groups of chunk N+1 are emitted
    interleaved into the attention instruction stream of chunk N, so the
    PE never idles waiting on the Act/DVE softmax chain.
  - First x-chunk DMA issued before weight DMAs (separate queues) to cut
    the startup stall.
"""
import numpy as np
import ml_dtypes

import concourse.bass as bass
import concourse.mybir as mybir
import concourse.tile as tile
from concourse import bacc
from concourse.bass_utils import run_bass_kernel_spmd

B, S, D = 4, 2048, 2048
H, DK = 16, 128
NCORES = 8
HPC = H // NCORES          # heads per core
CD = HPC * DK              # 256 contraction dims per core in out-proj
SC = 512                   # q/s chunk
NSC = S // SC              # 4
NST = SC // 128            # s-tiles per chunk
NDB = D // 256             # 8 d-pair-chunks (DoubleRow contraction tiles)
F32 = mybir.dt.float32
F16 = mybir.dt.float16
F8 = mybir.dt.float8e4
DR = mybir.MatmulPerfMode.DoubleRow
EXPF = mybir.ActivationFunctionType.Exp

WSCALE = 64.0              # host-side scale on Wq/Wk/Wv for fp8 range
SCALE = (1.0 / np.sqrt(DK)) / (WSCALE * WSCALE)
ASCALE = 16.0              # attn stored as 16*attn in fp8 hi/lo
WOS = 1024.0               # host-side scale on Wo for fp8 range
# bc = cst @ den: cst holds ASCALE/WSCALE so attps*rbc = ASCALE*attn
CSTV = WSCALE / ASCALE


def build_nc():
    nc = bacc.Bacc(None)
    # x hi/lo fp8: element (b, k, dcb, i, hl, s) = fp8_hl(x[b, s, dcb*256+i*128+k])
    xT = nc.dram_tensor("xT", [B, 128, NDB, 2, 2, S], F8, kind="ExternalInput")
    # wXa: (k, dcb, i, j, o) = Wh[dcb*256+i*128+k, o]  (duplicated over j)
    # wXb: (k, dcb, i, o)    = Wl[dcb*256+i*128+k, o]
    wqa = nc.dram_tensor("wqa", [128, NDB, 2, 2, CD], F8, kind="ExternalInput")
    wqb = nc.dram_tensor("wqb", [128, NDB, 2, CD], F8, kind="ExternalInput")
    wka = nc.dram_tensor("wka", [128, NDB, 2, 2, CD], F8, kind="ExternalInput")
    wkb = nc.dram_tensor("wkb", [128, NDB, 2, CD], F8, kind="ExternalInput")
    wva = nc.dram_tensor("wva", [128, NDB, 2, 2, CD], F8, kind="ExternalInput")
    wvb = nc.dram_tensor("wvb", [128, NDB, 2, CD], F8, kind="ExternalInput")
    # wo8[dk, hh, o] = Wo[o, hh*128+dk] * WOS, hi/lo fp8 parts
    wo8a = nc.dram_tensor("wo8a", [128, HPC, D], F8, kind="ExternalInput")
    wo8b = nc.dram_tensor("wo8b", [128, HPC, D], F8, kind="ExternalInput")
    # msk[k, j, q] = (j*128 + k) <= q  for the 4 diagonal tile offsets
    msk = nc.dram_tensor("msk", [128, NST, SC], F16, kind="ExternalInput")
    out = nc.dram_tensor("out", [B, S, D], F16, kind="ExternalOutput")

    with tile.TileContext(nc) as tc:
        with (
            tc.tile_pool(name="p_xs", bufs=2) as p_xs,
            tc.tile_pool(name="p_kv", bufs=2) as p_kv,
            tc.tile_pool(name="p_q", bufs=4) as p_q,
            tc.tile_pool(name="p_pt", bufs=2) as p_pt,
            tc.tile_pool(name="p_w", bufs=1) as p_w,
            tc.tile_pool(name="p_sm", bufs=2) as p_sm,
            tc.tile_pool(name="p_a8", bufs=2) as p_a8,
            tc.tile_pool(name="p_osb", bufs=2) as p_osb,
            tc.tile_pool(name="ps_m", bufs=4, space="PSUM") as ps_m,
            tc.tile_pool(name="ps_a", bufs=1, space="PSUM") as ps_a,
            tc.tile_pool(name="ps_bc", bufs=1, space="PSUM") as ps_bc,
            tc.tile_pool(name="ps_o", bufs=2, space="PSUM") as ps_o,
        ):
            # ---- weight / constant loads (gpsimd queue; x stream on sync) --
            wa_sb = {}
            wb_sb = {}
            wa_sb["q"] = p_w.tile([128, NDB, 2, 2, CD], F8, tag="wqa", name="wqa_sb")
            wb_sb["q"] = p_w.tile([128, NDB, 2, CD], F8, tag="wqb", name="wqb_sb")
            wa_sb["k"] = p_w.tile([128, NDB, 2, 2, CD], F8, tag="wka", name="wka_sb")
            wb_sb["k"] = p_w.tile([128, NDB, 2, CD], F8, tag="wkb", name="wkb_sb")
            wa_sb["v"] = p_w.tile([128, NDB, 2, 2, CD], F8, tag="wva", name="wva_sb")
            wb_sb["v"] = p_w.tile([128, NDB, 2, CD], F8, tag="wvb", name="wvb_sb")
            wo_sb = {}
            wo_sb["a"] = p_w.tile([128, HPC, D], F8, tag="wo8a", name="wo8a_sb")
            wo_sb["b"] = p_w.tile([128, HPC, D], F8, tag="wo8b", name="wo8b_sb")
            msk_sb = p_w.tile([128, NST, SC], F16, tag="msk")
            cst_sb = p_w.tile([128, 128], F16, tag="cst")
            nc.vector.memset(cst_sb, CSTV)

            xs_t = {}      # (b, sc) -> xs tile
            qT_t = {}      # (b, sc) -> qT tile
            kT_t = {}      # b -> kT tile
            v_t = {}       # b -> v tile

            def emit_xs(b, sc):
                xs = p_xs.tile([128, NDB, 2, 2, SC], F8, tag="xs")
                nc.sync.dma_start(
                    out=xs, in_=xT[b][:, :, :, :, sc * SC:(sc + 1) * SC])
                xs_t[(b, sc)] = xs

            def proj_units(b, sc):
                """8 emission closures: 4 QK groups + 4 V groups."""
                if b >= B:
                    return []
                units = []
                if b not in kT_t:
                    kT_t[b] = p_kv.tile([128, HPC, S], F16, tag="kT", name="kT")
                    v_t[b] = p_kv.tile([128, NSC * NST, CD], F16, tag="v", name="v_sb")
                kT = kT_t[b]
                v_sb = v_t[b]
                qT = p_q.tile([128, HPC, SC], F16, tag="qT")
                qT_t[(b, sc)] = qT

                def qk_group(h, wn):
                    def emit():
                        xs = xs_t[(b, sc)]
                        ps = ps_m.tile([128, SC], F32, tag="ps")
                        for dcb in range(NDB):
                            for i in range(2):
                                nc.tensor.matmul(
                                    ps,
                                    wa_sb[wn][:, dcb, i, :,
                                              h * DK:(h + 1) * DK],
                                    xs[:, dcb, i, :, :],
                                    start=(dcb == 0 and i == 0),
                                    stop=False,
                                    perf_mode=DR,
                                )
                        for dcb in range(NDB):
                            nc.tensor.matmul(
                                ps,
                                wb_sb[wn][:, dcb, :, h * DK:(h + 1) * DK],
                                xs[:, dcb, :, 0, :],
                                start=False,
                                stop=(dcb == NDB - 1),
                                perf_mode=DR,
                            )
                        if wn == "q":
                            nc.vector.tensor_copy(qT[:, h, :], ps)
                        else:
                            nc.vector.tensor_copy(
                                kT[:, h, sc * SC:(sc + 1) * SC], ps)
                    return emit

                def v_group(st):
                    def emit():
                        xs = xs_t[(b, sc)]
                        psv = ps_m.tile([128, SC], F32, tag="ps")
                        c0, c1 = st * 128, (st + 1) * 128
                        for dcb in range(NDB):
                            for i in range(2):
                                nc.tensor.matmul(
                                    psv[:, :CD],
                                    xs[:, dcb, i, :, c0:c1],
                                    wa_sb["v"][:, dcb, i, :, :],
                                    start=(dcb == 0 and i == 0),
                                    stop=False,
                                    perf_mode=DR,
                                )
                        for dcb in range(NDB):
                            nc.tensor.matmul(
                                psv[:, :CD],
                                xs[:, dcb, :, 0, c0:c1],
                                wb_sb["v"][:, dcb, :, :],
                                start=False,
                                stop=(dcb == NDB - 1),
                                perf_mode=DR,
                            )
                        nc.vector.tensor_copy(
                            v_sb[:, sc * NST + st, :], psv[:, :CD])
                    return emit

                for h in range(HPC):
                    units.append(qk_group(h, "q"))
                    units.append(qk_group(h, "k"))
                for st in range(NST):
                    units.append(v_group(st))
                return units

            def attn_head(b, c, h):
                """scores -> exp -> mask -> den(Pool) -> PV."""
                kT = kT_t[b]
                v_sb = v_t[b]
                qT = qT_t[(b, c)]
                nkt = 4 * c + 4
                pT = p_pt.tile([128, 16, SC], F16, tag="pT")
                den = p_sm.tile([128, SC], F16, tag="den")
                attps = ps_a.tile([128, SC], F32, tag="attps")
                for kt in range(nkt):
                    j = kt - 4 * c
                    qlo = j * 128 if j >= 0 else 0
                    sps = ps_m.tile([128, SC], F32, tag="ps")
                    nc.tensor.matmul(
                        sps[:, qlo:],
                        kT[:, h, kt * 128:(kt + 1) * 128],
                        qT[:, h, qlo:],
                        start=True, stop=True,
                    )
                    nc.scalar.activation(
                        pT[:, kt, qlo:], sps[:, qlo:], EXPF, scale=SCALE)
                    if j >= 0:
                        nc.vector.tensor_mul(
                            pT[:, kt, qlo:], pT[:, kt, qlo:],
                            msk_sb[:, j, qlo:])
                    if kt == 0:
                        nc.vector.tensor_copy(den, pT[:, 0, :])
                    else:
                        nc.vector.tensor_add(
                            den[:, qlo:], den[:, qlo:], pT[:, kt, qlo:])
                    nc.tensor.matmul(
                        attps[:, qlo:],
                        v_sb[:, kt, h * DK:(h + 1) * DK],
                        pT[:, kt, qlo:],
                        start=(kt == 0), stop=(kt == nkt - 1),
                        skip_group_check=(kt > 0),
                    )
                return den, attps

            def finish_head(den, attps):
                """bc ones-matmul + reciprocal + normalize: emitted a few
                proj units after the PV chain so the PE never waits on the
                Pool den accumulation."""
                bc = ps_bc.tile([128, SC], F32, tag="bc")
                nc.tensor.matmul(bc, cst_sb, den, start=True, stop=True)
                rbc = p_sm.tile([128, SC], F32, tag="rbc")
                t16 = p_sm.tile([128, SC], F16, tag="t16")
                nc.vector.reciprocal(rbc, bc)
                nc.vector.tensor_mul(t16, attps, rbc)
                return t16

            def attn_split(t16s, a8a, a8b):
                for h, t16 in enumerate(t16s):
                    nc.gpsimd.tensor_copy(a8a[:, h, :], t16)
                    nc.gpsimd.tensor_sub(a8b[:, h, :], t16, a8a[:, h, :])

            def outproj(b, c, a8a, a8b):
                for st in range(NST):
                    osb = p_osb.tile([128, NSC, SC], F16, tag="osb")
                    for oc in range(NSC):
                        ops = ps_o.tile([128, SC], F32, tag="ops")
                        os = slice(oc * SC, (oc + 1) * SC)
                        ts = slice(st * 128, (st + 1) * 128)
                        nc.tensor.matmul(ops, a8a[:, :, ts], wo_sb["a"][:, :, os],
                                         start=True, stop=False, perf_mode=DR)
                        nc.tensor.matmul(ops, a8b[:, :, ts], wo_sb["a"][:, :, os],
                                         start=False, stop=False, perf_mode=DR)
                        nc.tensor.matmul(ops, a8a[:, :, ts], wo_sb["b"][:, :, os],
                                         start=False, stop=True, perf_mode=DR)
                        if oc % 4 != 3:
                            nc.scalar.copy(osb[:, oc, :], ops)
                        else:
                            nc.vector.tensor_copy(osb[:, oc, :], ops)
                    nc.sync.dma_start(
                        out=out[b,
                                (c * NST + st) * 128:(c * NST + st + 1) * 128,
                                :],
                        in_=osb,
                    )

            # ---------------- schedule ----------------
            steps = [(b, c) for b in range(B) for c in range(NSC)]
            # startup: interleave first x chunk with weights in need-order
            xs0 = p_xs.tile([128, NDB, 2, 2, SC], F8, tag="xs", name="xs0")
            xs_t[(0, 0)] = xs0
            nc.sync.dma_start(out=wa_sb["q"], in_=wqa[:])
            for d in range(4):
                nc.sync.dma_start(
                    out=xs0[:, 2 * d:2 * d + 2],
                    in_=xT[0][:, 2 * d:2 * d + 2, :, :, 0:SC])
            nc.sync.dma_start(out=wb_sb["q"], in_=wqb[:])
            nc.sync.dma_start(out=wa_sb["k"], in_=wka[:])
            nc.sync.dma_start(out=wb_sb["k"], in_=wkb[:])
            nc.sync.dma_start(out=wa_sb["v"], in_=wva[:])
            nc.sync.dma_start(out=wb_sb["v"], in_=wvb[:])
            nc.sync.dma_start(out=msk_sb, in_=msk[:])
            nc.sync.dma_start(out=wo_sb["a"], in_=wo8a[:])
            nc.sync.dma_start(out=wo_sb["b"], in_=wo8b[:])
            emit_xs(0, 1)
            for u in proj_units(0, 0):
                u()
            for i, (b, c) in enumerate(steps):
                nb_, nc_ = steps[i + 1] if i + 1 < len(steps) else (B, 0)
                units = proj_units(nb_, nc_)
                # prefetch the x chunk one step ahead of its proj units
                pf = steps[i + 2] if i + 2 < len(steps) else None
                if pf is not None:
                    emit_xs(*pf)
                den0, att0 = attn_head(b, c, 0)
                for u in units[:2]:
                    u()
                t0 = finish_head(den0, att0)
                den1, att1 = attn_head(b, c, 1)
                for u in units[2:4]:
                    u()
                t1 = finish_head(den1, att1)
                a8a = p_a8.tile([128, HPC, SC], F8, tag="a8a")
                a8b = p_a8.tile([128, HPC, SC], F8, tag="a8b")
                attn_split((t0, t1), a8a, a8b)
                for u in units[4:6]:
                    u()
                outproj(b, c, a8a, a8b)
                for u in units[6:]:
                    u()
    nc.compile()
    return nc


def _fp8_split(a):
    hi = a.astype(ml_dtypes.float8_e4m3)
    lo = (a - hi.astype(np.float32)).astype(ml_dtypes.float8_e4m3)
    return hi, lo


def prepare_in_maps(x, Wq, Wk, Wv, Wo):
    x = np.asarray(x, dtype=np.float32)
    Wq = np.asarray(Wq, dtype=np.float32)
    Wk = np.asarray(Wk, dtype=np.float32)
    Wv = np.asarray(Wv, dtype=np.float32)
    Wo = np.asarray(Wo, dtype=np.float32)

    # x -> [B, 128, NDB, 2, 2, S] fp8 hi/lo
    xT = np.ascontiguousarray(x.transpose(0, 2, 1))  # [B, D, S]
    xh, xl = _fp8_split(xT)
    xh = xh.reshape(B, NDB, 2, 128, S)
    xl = xl.reshape(B, NDB, 2, 128, S)
    x8 = np.stack([xh, xl], axis=4)          # [B, NDB, 2, 128, 2, S]
    x8 = np.ascontiguousarray(x8.transpose(0, 3, 1, 2, 4, 5))

    qf = np.arange(SC)[None, None, :]
    kg = (np.arange(NST) * 128)[None, :, None] + np.arange(128)[:, None, None]
    msk = (kg <= qf).astype(np.float16)      # [128, NST, SC]

    in_maps = []
    for c in range(NCORES):
        r0, r1 = c * CD, (c + 1) * CD
        m = {"xT": x8, "msk": msk}
        for nm, W in (("q", Wq), ("k", Wk), ("v", Wv)):
            Wm = np.ascontiguousarray(W[r0:r1].T) * WSCALE   # [D, CD]
            hi, lo = _fp8_split(Wm)
            hi = hi.reshape(NDB, 2, 128, CD).transpose(2, 0, 1, 3)
            wa = np.ascontiguousarray(
                np.broadcast_to(hi[:, :, :, None, :], (128, NDB, 2, 2, CD)))
            wb = np.ascontiguousarray(
                lo.reshape(NDB, 2, 128, CD).transpose(2, 0, 1, 3))
            m[f"w{nm}a"] = wa
            m[f"w{nm}b"] = wb
        Wo_c = np.ascontiguousarray(Wo[:, r0:r1]) * WOS      # [D, CD]
        woh, wol = _fp8_split(Wo_c)
        # [D, CD] -> [128 dk, HPC, D]
        m["wo8a"] = np.ascontiguousarray(
            woh.reshape(D, HPC, 128).transpose(2, 1, 0))
        m["wo8b"] = np.ascontiguousarray(
            wol.reshape(D, HPC, 128).transpose(2, 1, 0))
        in_maps.append(m)
    return in_maps


_NC_CACHE = None


def kernel(x, Wq, Wk, Wv, Wo):
    global _NC_CACHE
    in_maps = prepare_in_maps(x, Wq, Wk, Wv, Wo)
    if _NC_CACHE is None:
        _NC_CACHE = build_nc()
    res = run_bass_kernel_spmd(_NC_CACHE, in_maps, list(range(NCORES)))
    total = res.results[0]["out"].astype(np.float32).copy()
    for i in range(1, NCORES):
        total += res.results[i]["out"].astype(np.float32)
    return total / (ASCALE * WOS)
